# revision 9
# baseline (speedup 1.0000x reference)
"""BBoxVAE Trainium2 kernel: 24-step conditional-VAE scan over B=65536.

Strategy: pure data parallel over 8 cores (8192 batch rows each).
Feature-major on-chip layout [features -> partitions, batch -> free dim];
every dense layer is one PE matmul out = W.T @ x with fp32 data in f32r
matmul mode (1 cycle/row).  All step-invariant algebra (label-MLP, the
one-hot conditioning MLP, linear e2*e3 composition, per-step bias tables)
is folded on the host into weights/bias tables; the sequential recurrence
runs fully on-chip with enc state resident in SBUF.
"""
import numpy as np

B = 65536
NCORES = 8
BC = B // NCORES          # 8192 batch rows per core
TN = 512                  # batch tile (matmul moving dim)
NT = BC // TN             # 16 tiles
T = 24                    # scan steps
HID = 128
LAT = 32
BBOX = 4
NL = 24

_CACHE = {}


def _build_nc():
    import concourse.bacc as bacc
    import concourse.tile as tile
    import concourse.mybir as mybir

    F32 = mybir.dt.float32
    F32R = mybir.dt.float32r
    AF = mybir.ActivationFunctionType
    ALU = mybir.AluOpType

    nc = bacc.Bacc()

    # --- per-core data inputs (host pre-transposed) ---
    ls_d = nc.dram_tensor("ls", [NL, BC], F32R, kind="ExternalInput")
    gt_d = nc.dram_tensor("gt", [T, BBOX, BC], F32R, kind="ExternalInput")
    eps_d = nc.dram_tensor("eps", [T, LAT, BC], F32, kind="ExternalInput")
    # --- folded weights (lhsT layout [fan_in, fan_out]) ---
    wshapes = {
        "wls1": [NL, HID], "wls2": [HID, HID], "wpb": [HID, HID],
        "wout3": [HID, HID], "wouta": [HID, HID], "we1": [BBOX, HID],
        "we23": [HID, LAT], "we3c": [HID, LAT], "wmulv": [LAT, 2 * LAT],
        "wd1c": [HID, HID], "wd1z": [LAT, HID], "wd2": [HID, 64],
        "wd3": [64, BBOX], "wu": [HID, 4 * HID], "wwb": [BBOX, 4 * HID],
    }
    wd = {k: nc.dram_tensor(k, s, F32R, kind="ExternalInput") for k, s in wshapes.items()}
    bshapes = {
        "bls1": [HID, 1], "bls2": [HID, 1], "bpb": [HID, 1], "be1": [HID, 1],
        "be3": [LAT, T], "blv": [LAT, 1], "bd1": [HID, T],
        "bd2": [64, 1], "bd3": [BBOX, 1], "gb": [HID, 4 * T],
    }
    bd = {k: nc.dram_tensor(k, s, F32, kind="ExternalInput") for k, s in bshapes.items()}
    out_d = nc.dram_tensor("out", [T, BBOX, BC], F32, kind="ExternalOutput")

    with tile.TileContext(nc) as tc:
        with (
            tc.tile_pool(name="const", bufs=1) as cp,
            tc.tile_pool(name="state", bufs=1) as st,
            tc.tile_pool(name="work", bufs=2) as wk,
            tc.tile_pool(name="io", bufs=3) as io,
            tc.tile_pool(name="psum", bufs=8, space="PSUM") as pp,
        )        :
            w = {}
            for k, s in wshapes.items():
                w[k] = cp.tile(s, F32R, tag=k, name=k)
                nc.sync.dma_start(w[k][:], wd[k][:])
            b = {}
            for k, s in bshapes.items():
                b[k] = cp.tile(s, F32, tag=k, name=k)
                nc.sync.dma_start(b[k][:], bd[k][:])

            # persistent state: enc + relu2 per batch tile
            enc = [st.tile([HID, TN], F32R, tag=f"enc{j}", name=f"enc{j}") for j in range(NT)]
            rl2 = [st.tile([HID, TN], F32R, tag=f"rl2{j}", name=f"rl2{j}") for j in range(NT)]
            for j in range(NT):
                nc.vector.memset(enc[j][:].bitcast(F32), 0.0)

            # ---- phase 0: label MLP (step-invariant), relu2 = relu(relu(ls@Wls1+b)@Wls2+b)
            for j in range(NT):
                sl = slice(j * TN, (j + 1) * TN)
                lst = io.tile([NL, TN], F32R, tag="lst")
                nc.sync.dma_start(lst[:], ls_d[:, sl])
                ps1 = pp.tile([HID, TN], F32, tag="ps")
                nc.tensor.matmul(ps1[:], w["wls1"][:], lst[:], start=True, stop=True)
                r1 = wk.tile([HID, TN], F32R, tag="r1")
                nc.scalar.activation(r1[:], ps1[:], AF.Relu, bias=b["bls1"][:], scale=1.0)
                ps2 = pp.tile([HID, TN], F32, tag="ps")
                nc.tensor.matmul(ps2[:], w["wls2"][:], r1[:], start=True, stop=True)
                nc.scalar.activation(rl2[j][:], ps2[:], AF.Relu, bias=b["bls2"][:], scale=1.0)

            # ---- main recurrence ----
            for t in range(T):
                for j in range(NT):
                    sl = slice(j * TN, (j + 1) * TN)
                    ej = enc[j]
                    # conditioning: i3 = relu(enc@Wpb + bpb)
                    ps_i3 = pp.tile([HID, TN], F32, tag="ps")
                    nc.tensor.matmul(ps_i3[:], w["wpb"][:], ej[:], start=True, stop=True)
                    i3 = wk.tile([HID, TN], F32R, tag="i3")
                    nc.vector.tensor_scalar(i3[:], ps_i3[:], b["bpb"][:], 0.0, ALU.add, ALU.max)
                    # cond_nb = i3@Wout3 + relu2@WoutA   (bias CB[t] folded downstream)
                    ps_cd = pp.tile([HID, TN], F32, tag="ps")
                    nc.tensor.matmul(ps_cd[:], w["wout3"][:], i3[:], start=True, stop=False)
                    nc.tensor.matmul(ps_cd[:], w["wouta"][:], rl2[j][:], start=False, stop=True)
                    cond = wk.tile([HID, TN], F32R, tag="cond")
                    nc.scalar.activation(cond[:], ps_cd[:], AF.Copy, bias=0.0, scale=1.0)
                    # encoder front: he1 = relu(gt@We1 + be1);   (be1 folded: kept explicit)
                    gtt = io.tile([BBOX, TN], F32R, tag="gtt")
                    nc.sync.dma_start(gtt[:], gt_d[t, :, sl])
                    ps_e1 = pp.tile([HID, TN], F32, tag="ps")
                    nc.tensor.matmul(ps_e1[:], w["we1"][:], gtt[:], start=True, stop=True)
                    he1 = wk.tile([HID, TN], F32R, tag="he1")
                    nc.vector.tensor_scalar(he1[:], ps_e1[:], b["be1"][:], 0.0, ALU.add, ALU.max)
                    # inter = relu(he1@We23 + cond@We3c + be3[t])
                    ps_e3 = pp.tile([LAT, TN], F32, tag="ps")
                    nc.tensor.matmul(ps_e3[:], w["we23"][:], he1[:], start=True, stop=False)
                    nc.tensor.matmul(ps_e3[:], w["we3c"][:], cond[:], start=False, stop=True)
                    inter = wk.tile([LAT, TN], F32R, tag="inter")
                    nc.scalar.activation(inter[:], ps_e3[:], AF.Relu, bias=b["be3"][:, t : t + 1], scale=1.0)
                    # mu/lv packed: [0:32]=mu_nb, [32:64]=lv_nb
                    ps_ml = pp.tile([2 * LAT, TN], F32, tag="ps")
                    nc.tensor.matmul(ps_ml[:], w["wmulv"][:], inter[:], start=True, stop=True)
                    sd = wk.tile([LAT, TN], F32, tag="sd")
                    nc.scalar.activation(sd[:], ps_ml[LAT : 2 * LAT, :], AF.Exp, bias=b["blv"][:], scale=0.5)
                    epst = io.tile([LAT, TN], F32, tag="epst")
                    nc.sync.dma_start(epst[:], eps_d[t, :, sl])
                    sde = wk.tile([LAT, TN], F32, tag="sde")
                    nc.gpsimd.tensor_tensor(sde[:], sd[:], epst[:], ALU.mult)
                    z = wk.tile([LAT, TN], F32R, tag="z")
                    nc.vector.tensor_tensor(z[:], sde[:], ps_ml[0:LAT, :], ALU.add)
                    # decoder: dh = relu(cond@Wd1c + z@Wd1z + bd1[t])
                    ps_d1 = pp.tile([HID, TN], F32, tag="ps")
                    nc.tensor.matmul(ps_d1[:], w["wd1c"][:], cond[:], start=True, stop=False)
                    nc.tensor.matmul(ps_d1[:], w["wd1z"][:], z[:], start=False, stop=True)
                    dh = wk.tile([HID, TN], F32R, tag="dh")
                    nc.vector.tensor_scalar(dh[:], ps_d1[:], b["bd1"][:, t : t + 1], 0.0, ALU.add, ALU.max)
                    # h2 = relu(dh@Wd2 + bd2)
                    ps_d2 = pp.tile([64, TN], F32, tag="ps")
                    nc.tensor.matmul(ps_d2[:], w["wd2"][:], dh[:], start=True, stop=True)
                    h2 = wk.tile([64, TN], F32R, tag="h2")
                    nc.vector.tensor_scalar(h2[:], ps_d2[:], b["bd2"][:], 0.0, ALU.add, ALU.max)
                    # bb = h2@Wd3 + bd3  -> output + lstm input
                    ps_d3 = pp.tile([BBOX, TN], F32, tag="ps")
                    nc.tensor.matmul(ps_d3[:], w["wd3"][:], h2[:], start=True, stop=True)
                    bb = wk.tile([BBOX, TN], F32R, tag="bb")
                    nc.scalar.activation(bb[:], ps_d3[:], AF.Identity, bias=b["bd3"][:], scale=1.0)
                    nc.sync.dma_start(out_d[t, :, sl], bb[:].bitcast(F32))
                    # lstm gates: z_g = enc@U_g + bb@Wb_g + gbias[t]  (g: 0=i,1=f,2=c,3=o)
                    gate = []
                    for g in range(4):
                        ps_g = pp.tile([HID, TN], F32, tag="ps")
                        nc.tensor.matmul(ps_g[:], w["wu"][:, g * HID : (g + 1) * HID], ej[:], start=True, stop=False)
                        nc.tensor.matmul(ps_g[:], w["wwb"][:, g * HID : (g + 1) * HID], bb[:], start=False, stop=True)
                        gate.append(ps_g)
                    # i,f,o: tanh(0.5 z_g + 0.5 bias) ; c: tanh(z_c + bias)
                    tg = []
                    for g, scale in ((0, 0.5), (1, 0.5), (2, 1.0), (3, 0.5)):
                        tt = wk.tile([HID, TN], F32, tag=f"tg{g}", name=f"tg{g}")
                        nc.scalar.activation(tt[:], gate[g][:], AF.Tanh, bias=b["gb"][:, g * T + t : g * T + t + 1], scale=scale)
                        tg.append(tt)
                    # c_new = sig(zf)*enc + sig(zi)*tanh(zc);  sig via (0.5*t+0.5)
                    aa = wk.tile([HID, TN], F32, tag="aa")
                    ac1 = wk.tile([HID, 1], F32, tag="ac1")
                    nc.vector.affine_mul_reduce(aa[:], ac1[:], tg[1][:], ej[:], 0.5, 0.5)
                    bb2 = wk.tile([HID, TN], F32, tag="bb2")
                    ac2 = wk.tile([HID, 1], F32, tag="ac2")
                    nc.vector.affine_mul_reduce(bb2[:], ac2[:], tg[0][:], tg[2][:], 0.5, 0.5)
                    cnew = wk.tile([HID, TN], F32, tag="cnew")
                    nc.gpsimd.tensor_tensor(cnew[:], aa[:], bb2[:], ALU.add)
                    tcn = wk.tile([HID, TN], F32, tag="tcn")
                    nc.scalar.activation(tcn[:], cnew[:], AF.Tanh, bias=0.0, scale=1.0)
                    ac3 = wk.tile([HID, 1], F32, tag="ac3")
                    nc.vector.affine_mul_reduce(ej[:], ac3[:], tg[3][:], tcn[:], 0.5, 0.5)
    nc.finalize()
    return nc


def _host_prepare(label_set, bbox_input, eps, params):
    f32 = np.float32
    p = {k: np.ascontiguousarray(np.asarray(v, f32)) for k, v in params.items()}
    relu = lambda x: np.maximum(x, 0.0)

    I2 = relu(relu(np.eye(NL, dtype=f32) @ p["c_cl1_W"] + p["c_cl1_b"]) @ p["c_cl2_W"] + p["c_cl2_b"])
    CB = I2 @ p["c_out_W"][HID : 2 * HID] + p["c_out_b"]          # [24,128]
    W_e3h = p["e3_W"][:HID]
    W_e3c = p["e3_W"][HID:]
    weights = {
        "wls1": p["c_ls1_W"], "wls2": p["c_ls2_W"], "wpb": p["c_pb_W"],
        "wout3": p["c_out_W"][2 * HID :], "wouta": p["c_out_W"][:HID],
        "we1": p["e1_W"], "we23": p["e2_W"] @ W_e3h, "we3c": W_e3c,
        "wmulv": np.concatenate([p["e_mu_W"], p["e_lv_W"]], axis=1),
        "wd1c": p["d1_W"][:HID], "wd1z": p["d1_W"][HID:],
        "wd2": p["d2_W"], "wd3": p["d3_W"], "wu": p["lstm_U"],
        "wwb": p["lstm_W"][NL:],
    }
    gbias = p["lstm_W"][:NL] + p["lstm_b"]                          # [24,512]
    gb = gbias.copy()
    gb[:, 0:HID] *= 0.5          # i
    gb[:, HID : 2 * HID] *= 0.5  # f
    gb[:, 3 * HID :] *= 0.5      # o
    # pack as [128, 4*24]: column g*24+t holds gate g's bias at step t
    gb_pack = np.zeros((HID, 4 * T), f32)
    for g in range(4):
        gb_pack[:, g * T : (g + 1) * T] = gb[:, g * HID : (g + 1) * HID].T
    biases = {
        "bls1": p["c_ls1_b"][:, None], "bls2": p["c_ls2_b"][:, None],
        "bpb": p["c_pb_b"][:, None],
        "be1": p["e1_b"][:, None],
        "be3": (p["e3_b"] + p["e2_b"] @ W_e3h + CB @ W_e3c).T,       # [32,24]
        "blv": 0.5 * p["e_lv_b"][:, None],
        "bd1": (p["d1_b"] + CB @ p["d1_W"][:HID] + p["e_mu_b"] @ p["d1_W"][HID:]).T,  # [128,24]
        "bd2": p["d2_b"][:, None], "bd3": p["d3_b"][:, None],
        "gb": gb_pack,
    }

    ls = np.asarray(label_set, f32).reshape(NCORES, BC, NL).transpose(0, 2, 1)
    gt = np.asarray(bbox_input, f32).reshape(NCORES, BC, T, BBOX).transpose(0, 2, 3, 1)
    ep = np.asarray(eps, f32).reshape(NCORES, BC, T, LAT).transpose(0, 2, 3, 1)

    in_maps = []
    for c in range(NCORES):
        m = {"ls": np.ascontiguousarray(ls[c]), "gt": np.ascontiguousarray(gt[c]),
             "eps": np.ascontiguousarray(ep[c])}
        for k, v in weights.items():
            m[k] = np.ascontiguousarray(v)
        for k, v in biases.items():
            m[k] = np.ascontiguousarray(np.asarray(v, f32))
        in_maps.append(m)
    return in_maps


def get_nc():
    if "nc" not in _CACHE:
        _CACHE["nc"] = _build_nc()
    return _CACHE["nc"]


def kernel(label_set, bbox_input, eps, params):
    from concourse import bass2jax

    in_maps = _host_prepare(label_set, bbox_input, eps, params)
    nc = get_nc()
    results = bass2jax.run_bass_via_pjrt(nc, in_maps, n_cores=NCORES)
    outs = np.stack([r["out"] for r in results])                   # [8,24,4,BC]
    return np.ascontiguousarray(outs.transpose(0, 3, 1, 2).reshape(B, T, BBOX))


# revision 16
# speedup vs baseline: 1913.2036x; 1913.2036x over previous
"""BBoxVAE Trainium2 kernel: 24-step conditional-VAE scan over B=65536.

Strategy: pure data parallel over 8 cores (8192 batch rows each).
Feature-major on-chip layout [features -> partitions, batch -> free dim];
every dense layer is one PE matmul out = W.T @ x with fp32 data in f32r
matmul mode (1 cycle/row).  All step-invariant algebra (label-MLP, the
one-hot conditioning MLP, linear e2*e3 composition, per-step bias tables)
is folded on the host into weights/bias tables; the sequential recurrence
runs fully on-chip with enc state resident in SBUF.
"""
import numpy as np

B = 65536
NCORES = 8
BC = B // NCORES          # 8192 batch rows per core
TN = 512                  # batch tile (matmul moving dim)
NT = BC // TN             # 16 tiles
T = 24                    # scan steps
HID = 128
LAT = 32
BBOX = 4
NL = 24

_CACHE = {}


def _build_nc():
    import concourse.bacc as bacc
    import concourse.tile as tile
    import concourse.mybir as mybir

    F32 = mybir.dt.float32
    F32R = mybir.dt.float32r
    AF = mybir.ActivationFunctionType
    ALU = mybir.AluOpType

    nc = bacc.Bacc()

    # --- per-core data inputs (host pre-transposed) ---
    ls_d = nc.dram_tensor("ls", [NL, BC], F32R, kind="ExternalInput")
    gt_d = nc.dram_tensor("gt", [T, BBOX, BC], F32R, kind="ExternalInput")
    eps_d = nc.dram_tensor("eps", [T, LAT, BC], mybir.dt.bfloat16, kind="ExternalInput")
    # --- folded weights (lhsT layout [fan_in, fan_out]) ---
    wshapes = {
        "wls1": [NL, HID], "wls2": [HID, HID], "wpb": [HID, HID],
        "we1": [BBOX, HID],
        "we23": [HID, LAT], "we3i": [HID, LAT], "we3r": [HID, LAT],
        "wmulv": [LAT, 2 * LAT],
        "wd1i": [HID, HID], "wd1r": [HID, HID], "wd1z": [LAT, HID], "wd2": [HID, 64],
        "wd3": [64, BBOX], "wu": [HID, 4 * HID], "wwb": [BBOX, 4 * HID],
    }
    wd = {k: nc.dram_tensor(k, s, F32R, kind="ExternalInput") for k, s in wshapes.items()}
    bshapes = {
        "bls1": [HID, 1], "bls2": [HID, 1], "bpb": [HID, 1], "be1": [HID, 1],
        "be3": [LAT, T], "blv": [LAT, 1], "bd1": [HID, T],
        "bd2": [64, 1], "bd3": [BBOX, 1], "gb": [HID, 4 * T],
    }
    bd = {k: nc.dram_tensor(k, s, F32, kind="ExternalInput") for k, s in bshapes.items()}
    out_d = nc.dram_tensor("out", [T, BBOX, BC], F32, kind="ExternalOutput")

    with tile.TileContext(nc) as tc:
        with (
            tc.tile_pool(name="const", bufs=1) as cp,
            tc.tile_pool(name="state", bufs=1) as st,
            tc.tile_pool(name="work", bufs=2) as wk,
            tc.tile_pool(name="io", bufs=3) as io,
            tc.tile_pool(name="psum", bufs=8, space="PSUM") as pp,
        )        :
            w = {}
            for k, s in wshapes.items():
                w[k] = cp.tile(s, F32R, tag=k, name=k)
                nc.sync.dma_start(w[k][:], wd[k][:])
            b = {}
            for k, s in bshapes.items():
                b[k] = cp.tile(s, F32, tag=k, name=k)
                nc.sync.dma_start(b[k][:], bd[k][:])

            # persistent state: enc + relu2 per batch tile
            enc = [st.tile([HID, TN], F32R, tag=f"enc{j}", name=f"enc{j}") for j in range(NT)]
            rl2 = [st.tile([HID, TN], F32R, tag=f"rl2{j}", name=f"rl2{j}") for j in range(NT)]
            for j in range(NT):
                nc.vector.memset(enc[j][:].bitcast(F32), 0.0)

            # ---- phase 0: label MLP (step-invariant), relu2 = relu(relu(ls@Wls1+b)@Wls2+b)
            for j in range(NT):
                sl = slice(j * TN, (j + 1) * TN)
                lst = io.tile([NL, TN], F32R, tag="lst")
                nc.sync.dma_start(lst[:], ls_d[:, sl])
                ps1 = pp.tile([HID, TN], F32, tag="ps")
                nc.tensor.matmul(ps1[:], w["wls1"][:], lst[:], start=True, stop=True)
                r1 = wk.tile([HID, TN], F32R, tag="r1")
                nc.scalar.activation(r1[:], ps1[:], AF.Relu, bias=b["bls1"][:], scale=1.0)
                ps2 = pp.tile([HID, TN], F32, tag="ps")
                nc.tensor.matmul(ps2[:], w["wls2"][:], r1[:], start=True, stop=True)
                nc.scalar.activation(rl2[j][:], ps2[:], AF.Relu, bias=b["bls2"][:], scale=1.0)

            # ---- main recurrence ----
            for t in range(T):
                for j in range(NT):
                    sl = slice(j * TN, (j + 1) * TN)
                    ej = enc[j]
                    # conditioning: i3 = relu(enc@Wpb + bpb)
                    ps_i3 = pp.tile([HID, TN], F32, tag="ps")
                    nc.tensor.matmul(ps_i3[:], w["wpb"][:], ej[:], start=True, stop=True)
                    i3 = wk.tile([HID, TN], F32R, tag="i3")
                    nc.vector.tensor_scalar(i3[:], ps_i3[:], b["bpb"][:], 0.0, ALU.add, ALU.max)
                    # cond_nb = i3@Wout3 + relu2@WoutA   (bias CB[t] folded downstream)
                    ps_cd = pp.tile([HID, TN], F32, tag="ps")
                    nc.tensor.matmul(ps_cd[:], w["wout3"][:], i3[:], start=True, stop=False)
                    nc.tensor.matmul(ps_cd[:], w["wouta"][:], rl2[j][:], start=False, stop=True)
                    cond = wk.tile([HID, TN], F32R, tag="cond")
                    nc.scalar.activation(cond[:], ps_cd[:], AF.Copy, bias=0.0, scale=1.0)
                    # encoder front: he1 = relu(gt@We1 + be1);   (be1 folded: kept explicit)
                    gtt = io.tile([BBOX, TN], F32R, tag="gtt")
                    nc.sync.dma_start(gtt[:], gt_d[t, :, sl])
                    ps_e1 = pp.tile([HID, TN], F32, tag="ps")
                    nc.tensor.matmul(ps_e1[:], w["we1"][:], gtt[:], start=True, stop=True)
                    he1 = wk.tile([HID, TN], F32R, tag="he1")
                    nc.vector.tensor_scalar(he1[:], ps_e1[:], b["be1"][:], 0.0, ALU.add, ALU.max)
                    # inter = relu(he1@We23 + cond@We3c + be3[t])
                    ps_e3 = pp.tile([LAT, TN], F32, tag="ps")
                    nc.tensor.matmul(ps_e3[:], w["we23"][:], he1[:], start=True, stop=False)
                    nc.tensor.matmul(ps_e3[:], w["we3c"][:], cond[:], start=False, stop=True)
                    inter = wk.tile([LAT, TN], F32R, tag="inter")
                    nc.scalar.activation(inter[:], ps_e3[:], AF.Relu, bias=b["be3"][:, t : t + 1], scale=1.0)
                    # mu/lv packed: [0:32]=mu_nb, [32:64]=lv_nb
                    ps_ml = pp.tile([2 * LAT, TN], F32, tag="ps")
                    nc.tensor.matmul(ps_ml[:], w["wmulv"][:], inter[:], start=True, stop=True)
                    sd = wk.tile([LAT, TN], F32, tag="sd")
                    nc.scalar.activation(sd[:], ps_ml[LAT : 2 * LAT, :], AF.Exp, bias=b["blv"][:], scale=0.5)
                    epst = io.tile([LAT, TN], F32, tag="epst")
                    nc.sync.dma_start(epst[:], eps_d[t, :, sl])
                    sde = wk.tile([LAT, TN], F32, tag="sde")
                    nc.gpsimd.tensor_tensor(sde[:], sd[:], epst[:], ALU.mult)
                    z = wk.tile([LAT, TN], F32R, tag="z")
                    nc.vector.tensor_tensor(z[:], sde[:], ps_ml[0:LAT, :], ALU.add)
                    # decoder: dh = relu(cond@Wd1c + z@Wd1z + bd1[t])
                    ps_d1 = pp.tile([HID, TN], F32, tag="ps")
                    nc.tensor.matmul(ps_d1[:], w["wd1c"][:], cond[:], start=True, stop=False)
                    nc.tensor.matmul(ps_d1[:], w["wd1z"][:], z[:], start=False, stop=True)
                    dh = wk.tile([HID, TN], F32R, tag="dh")
                    nc.vector.tensor_scalar(dh[:], ps_d1[:], b["bd1"][:, t : t + 1], 0.0, ALU.add, ALU.max)
                    # h2 = relu(dh@Wd2 + bd2)
                    ps_d2 = pp.tile([64, TN], F32, tag="ps")
                    nc.tensor.matmul(ps_d2[:], w["wd2"][:], dh[:], start=True, stop=True)
                    h2 = wk.tile([64, TN], F32R, tag="h2")
                    nc.vector.tensor_scalar(h2[:], ps_d2[:], b["bd2"][:], 0.0, ALU.add, ALU.max)
                    # bb = h2@Wd3 + bd3  -> output + lstm input
                    ps_d3 = pp.tile([BBOX, TN], F32, tag="ps")
                    nc.tensor.matmul(ps_d3[:], w["wd3"][:], h2[:], start=True, stop=True)
                    bb = wk.tile([BBOX, TN], F32R, tag="bb")
                    nc.scalar.activation(bb[:], ps_d3[:], AF.Identity, bias=b["bd3"][:], scale=1.0)
                    nc.sync.dma_start(out_d[t, :, sl], bb[:].bitcast(F32))
                    # lstm gates: z_g = enc@U_g + bb@Wb_g + gbias[t]  (g: 0=i,1=f,2=c,3=o)
                    gate = []
                    for g in range(4):
                        ps_g = pp.tile([HID, TN], F32, tag="ps")
                        nc.tensor.matmul(ps_g[:], w["wu"][:, g * HID : (g + 1) * HID], ej[:], start=True, stop=False)
                        nc.tensor.matmul(ps_g[:], w["wwb"][:, g * HID : (g + 1) * HID], bb[:], start=False, stop=True)
                        gate.append(ps_g)
                    # i,f,o: tanh(0.5 z_g + 0.5 bias) ; c: tanh(z_c + bias)
                    tg = []
                    for g, scale in ((0, 0.5), (1, 0.5), (2, 1.0), (3, 0.5)):
                        tt = wk.tile([HID, TN], F32, tag=f"tg{g}", name=f"tg{g}")
                        nc.scalar.activation(tt[:], gate[g][:], AF.Tanh, bias=b["gb"][:, g * T + t : g * T + t + 1], scale=scale)
                        tg.append(tt)
                    # c_new = sig(zf)*enc + sig(zi)*tanh(zc);  sig via (0.5*t+0.5)
                    aa = wk.tile([HID, TN], F32, tag="aa")
                    ac1 = wk.tile([HID, 1], F32, tag="ac1")
                    nc.vector.affine_mul_reduce(aa[:], ac1[:], tg[1][:], ej[:], 0.5, 0.5)
                    bb2 = wk.tile([HID, TN], F32, tag="bb2")
                    ac2 = wk.tile([HID, 1], F32, tag="ac2")
                    nc.vector.affine_mul_reduce(bb2[:], ac2[:], tg[0][:], tg[2][:], 0.5, 0.5)
                    cnew = wk.tile([HID, TN], F32, tag="cnew")
                    nc.gpsimd.tensor_tensor(cnew[:], aa[:], bb2[:], ALU.add)
                    tcn = wk.tile([HID, TN], F32, tag="tcn")
                    nc.scalar.activation(tcn[:], cnew[:], AF.Tanh, bias=0.0, scale=1.0)
                    ac3 = wk.tile([HID, 1], F32, tag="ac3")
                    nc.vector.affine_mul_reduce(ej[:], ac3[:], tg[3][:], tcn[:], 0.5, 0.5)
    nc.finalize()
    return nc


def _host_prepare(label_set, bbox_input, eps, params):
    f32 = np.float32
    p = {k: np.ascontiguousarray(np.asarray(v, f32)) for k, v in params.items()}
    relu = lambda x: np.maximum(x, 0.0)

    I2 = relu(relu(np.eye(NL, dtype=f32) @ p["c_cl1_W"] + p["c_cl1_b"]) @ p["c_cl2_W"] + p["c_cl2_b"])
    CB = I2 @ p["c_out_W"][HID : 2 * HID] + p["c_out_b"]          # [24,128]
    W_e3h = p["e3_W"][:HID]
    W_e3c = p["e3_W"][HID:]
    Wout3 = p["c_out_W"][2 * HID :]
    WoutA = p["c_out_W"][:HID]
    Wd1c = p["d1_W"][:HID]
    weights = {
        "wls1": p["c_ls1_W"], "wls2": p["c_ls2_W"], "wpb": p["c_pb_W"],
        "we1": p["e1_W"], "we23": p["e2_W"] @ W_e3h,
        "we3i": Wout3 @ W_e3c, "we3r": WoutA @ W_e3c,
        "wmulv": np.concatenate([p["e_mu_W"], p["e_lv_W"]], axis=1),
        "wd1i": Wout3 @ Wd1c, "wd1r": WoutA @ Wd1c, "wd1z": p["d1_W"][HID:],
        "wd2": p["d2_W"], "wd3": p["d3_W"], "wu": p["lstm_U"],
        "wwb": p["lstm_W"][NL:],
    }
    gbias = p["lstm_W"][:NL] + p["lstm_b"]                          # [24,512]
    gb = gbias.copy()
    gb[:, 0:HID] *= 0.5          # i
    gb[:, HID : 2 * HID] *= 0.5  # f
    gb[:, 3 * HID :] *= 0.5      # o
    # pack as [128, 4*24]: column g*24+t holds gate g's bias at step t
    gb_pack = np.zeros((HID, 4 * T), f32)
    for g in range(4):
        gb_pack[:, g * T : (g + 1) * T] = gb[:, g * HID : (g + 1) * HID].T
    biases = {
        "bls1": p["c_ls1_b"][:, None], "bls2": p["c_ls2_b"][:, None],
        "bpb": p["c_pb_b"][:, None],
        "be1": p["e1_b"][:, None],
        "be3": (p["e3_b"] + p["e2_b"] @ W_e3h + CB @ W_e3c).T,       # [32,24]
        "blv": 0.5 * p["e_lv_b"][:, None],
        "bd1": (p["d1_b"] + CB @ Wd1c + p["e_mu_b"] @ p["d1_W"][HID:]).T,  # [128,24]
        "bd2": p["d2_b"][:, None], "bd3": p["d3_b"][:, None],
        "gb": gb_pack,
    }

    ls = np.asarray(label_set, f32).reshape(NCORES, BC, NL).transpose(0, 2, 1)
    gt = np.asarray(bbox_input, f32).reshape(NCORES, BC, T, BBOX).transpose(0, 2, 3, 1)
    import ml_dtypes
    ep = np.asarray(eps, f32).reshape(NCORES, BC, T, LAT).transpose(0, 2, 3, 1).astype(ml_dtypes.bfloat16)

    in_maps = []
    for c in range(NCORES):
        m = {"ls": np.ascontiguousarray(ls[c]), "gt": np.ascontiguousarray(gt[c]),
             "eps": np.ascontiguousarray(ep[c])}
        for k, v in weights.items():
            m[k] = np.ascontiguousarray(v)
        for k, v in biases.items():
            m[k] = np.ascontiguousarray(np.asarray(v, f32))
        in_maps.append(m)
    return in_maps


def get_nc():
    if "nc" not in _CACHE:
        _CACHE["nc"] = _build_nc()
    return _CACHE["nc"]


def kernel(label_set, bbox_input, eps, params):
    from concourse import bass2jax

    in_maps = _host_prepare(label_set, bbox_input, eps, params)
    nc = get_nc()
    results = bass2jax.run_bass_via_pjrt(nc, in_maps, n_cores=NCORES)
    outs = np.stack([r["out"] for r in results])                   # [8,24,4,BC]
    return np.ascontiguousarray(outs.transpose(0, 3, 1, 2).reshape(B, T, BBOX))
